# revision 2
# baseline (speedup 1.0000x reference)
"""Trainium2 Bass kernel for the pairwise-KL contrastive loss (nn_KL_Loss).

Reference math (N=512, D=128, 2N=1024):
    mu  = concat(p1_loc, p2_loc)     [2N, D]
    var = concat(p1_scale, p2_scale) [2N, D]
    kld[i,j] = 0.5 * sum_d( lv[j]-lv[i]-1 + ((mu[i]-mu[j])^2 + var[i])/var[j] )
    sim = where(diag, -9e6, kld) * T          (T = 0.01)
    loss = mean_i( sim[i, (i+N)%2N] - logsumexp_j sim[i,:] )

Kernel decomposition (one 128-row block per core):
    2*kld[i,j] = R[i,j] - L[i] - D,  where
    R[i,j] = sum_d A[i,d]*iv[j,d] - 2*sum_d mu[i,d]*(mu*iv)[j,d]
             + sum_d (mu^2*iv)[j,d] + sum_d lv[j,d]
    (A = mu^2 + var, iv = 1/var, lv = log var, L[i] = sum_d lv[i,d])
    -> 4 TensorE matmuls (K = D = 128) accumulated in PSUM per 512-col half.

    The per-row shift -c*(L[i]+D) cancels in sim_pos - logsumexp, so with
    c = 0.5*T:   loss_i = c*R[i,pos] - log( sum_j exp(c*R[i,j]) - exp(c*(L[i]+D)) )
    The subtracted term removes the diagonal (self) entry exactly
    (R[i,i] = L[i]+D).  sim values are O(1) (max ~2.7) so fp32 sum-of-exps
    is stable without max-subtraction.

Layout strategy (all data prep on HOST, which the contract allows --
sharding/gather happen inside kernel()):
  * Inputs are pre-TRANSPOSED on host to [D, 2N] = [128, 1024] so the
    matmul operands (contraction over d on the partition axis) stream
    straight from DRAM -> SBUF with NO on-chip transposes at all.
  * Per core c the columns are rotated by -128c and permuted to
    [own(0:128) | pos-block | rest], so every core runs the identical
    program: self-pairs are the diagonal of columns 0:128 and positive
    pairs the diagonal of columns 128:256 (both in PSUM half 1).
  * Output is a [1, 128] row (one partition, one DMA descriptor) --
    a [128, 1] column would be 128 4-byte descriptors whose completion
    semaphore costs ~6 us extra at kernel end.

Perf notes:
  * PE DVFS: the tensor engine starts at ~1.54 ns/cycle and only ramps
    to 0.42 ns/cycle after ~3 us of continuous work, so a string of
    dummy matmuls during the input-DMA window pre-ramps the clock.
  * DMA issue instructions cost ~0.7-1.0 us of queue time each, so the
    four input loads are spread over the sync and gpsimd queues, and
    the scalar (ACT) queue is kept free of DMA work so the activation
    table load + Ln chain can start immediately.
"""

import sys
import types

for _p in ("/opt/trn_rl_repo", "/opt/trn_rl_repo/concourse"):
    if _p not in sys.path:
        sys.path.insert(0, _p)

import numpy as np

import bass_rust as _bass_rust
import concourse.bacc as bacc
import concourse.bass as bass  # noqa: F401  (AP helpers)
import concourse.tile as tile
from concourse import mybir
from concourse.bass_utils import run_bass_kernel_spmd
from concourse.hw_specs import get_activation_tables

F32 = mybir.dt.float32
F32R = mybir.dt.float32r
AF = mybir.ActivationFunctionType
ALU = mybir.AluOpType

N2 = 1024  # 2N rows
D = 128
TEMP = 0.01
C = 0.5 * TEMP  # 0.005
N_CORES = 8
N_DUMMY = 10  # PE warm-up matmuls (DVFS ramp) during the input DMA window

_CACHED_NC = None


def _patched_act_table_loads(self):
    """insert_act_table_loads steered so Exp and Ln resolve to the one set
    that has both (`natural_log_exp_and_others`) -> a single ACT_TABLE_LOAD
    instead of thrashing between `exp_and_others` and `natural_log` (~1.3us
    per reload).  The list ORDER must stay untouched (act_func_set_id is the
    index into act_info.json), so instead of reordering we strip Exp/Ln from
    every other set's function list."""
    has_activation = any(
        isinstance(i, mybir.InstActivation)
        for b in self.main_func.blocks
        for i in b.instructions
    )
    if not has_activation:
        return
    keep = "natural_log_exp_and_others"
    tables = [
        (name,
         funcs if name == keep
         else {f for f in funcs if f not in (AF.Exp, AF.Ln)})
        for name, funcs in get_activation_tables(self.m.arch).items()
    ]
    _bass_rust.insert_act_table_loads(self, tables)


def _recip_approx_fast_f32r(nc, out, in_):
    """reciprocal_approx_fast with a float32r-typed output tile.  The wrapper
    in bass asserts fp32 in AND out, but only the *input* needs the fp32 bit
    layout (BITWISE_NOT exponent-flip seed); the output write is a normal DVE
    store which rounds to the out AP's dtype."""
    from concourse.dve_ops import RECIP_APPROX_FAST_CONSTS, RECIPROCAL_APPROX_FAST

    c = RECIP_APPROX_FAST_CONSTS
    return nc.vector._custom_dve(
        RECIPROCAL_APPROX_FAST, out=out, in0=in_,
        s0=c["s0"], s1=c["s1"], imm2=c["imm2"])


def build_nc(loop_n=None):
    from contextlib import nullcontext

    nc = bacc.Bacc(None, target_bir_lowering=False, debug=False)
    nc.insert_act_table_loads = types.MethodType(_patched_act_table_loads, nc)

    # Host supplies transposed + per-core-permuted inputs: [D, 2N].
    mu_d = nc.dram_tensor("muT", [D, N2], F32, kind="ExternalInput")
    var_d = nc.dram_tensor("varT", [D, N2], F32, kind="ExternalInput")
    loss_d = nc.dram_tensor("loss", [1, 128], F32, kind="ExternalOutput")

    with tile.TileContext(nc) as tc:
        with (
            tc.tile_pool(name="consts", bufs=1) as consts,
            tc.tile_pool(name="nat", bufs=1) as nat,
            tc.tile_pool(name="big", bufs=1) as big,
            tc.tile_pool(name="small", bufs=1) as small,
            tc.tile_pool(name="psum", bufs=1, space="PSUM") as psum,
        ):
            loop_cm = tc.For_i(0, loop_n, 1) if loop_n else nullcontext()
            with loop_cm:
                body(nc, tc, consts, nat, big, small, psum,
                     mu_d, var_d, loss_d)

    nc.compile()
    return nc


def body(nc, tc, consts, nat, big, small, psum, mu_d, var_d, loss_d):
    # ---- constants ----
    # GPSIMD queue: ones memset first (unblocks ones128 -> PE dummies),
    # then the two mu DMA issues (data wanted early), then ident/cd_bias
    # (not needed until ~5-6us).
    ones_f32 = consts.tile([128, 128], F32)
    nc.gpsimd.memset(ones_f32, 1.0)

    # ---- input DMA ----
    # Half 1 = permuted cols 0:512 (own block + positive-pair block),
    # half 2 = cols 512:1024.  var on the sync HWDGE queue, mu on the
    # gpsimd queue; the scalar queue stays DMA-free so the ACT table
    # load + Ln chain can start at t~0.
    vt = nat.tile([128, N2], F32)
    mt = nat.tile([128, N2], F32)
    nc.sync.dma_start(out=vt[:, 0:512], in_=var_d[:, 0:512])
    nc.sync.dma_start(out=vt[:, 512:1024], in_=var_d[:, 512:1024])
    nc.gpsimd.dma_start(out=mt[:, 0:512], in_=mu_d[:, 0:512])
    nc.gpsimd.dma_start(out=mt[:, 512:1024], in_=mu_d[:, 512:1024])

    ones128 = consts.tile([128, 128], F32R)
    nc.vector.tensor_copy(ones128, ones_f32)
    ones_col = consts.tile([128, 1], F32R)
    nc.vector.tensor_copy(ones_col, ones_f32[:, 0:1])
    ident = consts.tile([128, 128], F32)
    # iota[p, x] = p - x ; == 0 on the diagonal
    nc.gpsimd.affine_select(
        out=ident,
        in_=ones_f32,
        pattern=[[-1, 128]],
        base=0,
        channel_multiplier=1,
        compare_op=ALU.is_equal,
        fill=0.0,
    )
    cd_bias = consts.tile([128, 1], F32)
    nc.gpsimd.memset(cd_bias, float(C * D))
    # ACT warm-up: trigger the (single) exp+ln table load at t~0 so it
    # overlaps the input DMA instead of stalling the first real Ln.
    warm = consts.tile([128, 1], F32)
    nc.scalar.activation(warm, ones_col, AF.Ln)

    # ---- PSUM ----
    p_R1 = psum.tile([128, 512], F32)
    p_R2 = psum.tile([128, 512], F32)
    p_L = psum.tile([128, 1], F32)
    p_lossT = psum.tile([1, 128], F32)
    p_dummy = psum.tile([128, 128], F32)

    # ---- PE warm-up: ramp the tensor-engine clock during the DMA wait ----
    for _ in range(N_DUMMY):
        nc.tensor.matmul(p_dummy, ones128, ones128, start=True, stop=True)

    # ---- derived per-column tensors (all SBUF -> SBUF / DVE+ACT+GP) ----
    lv1 = big.tile([128, 512], F32R)
    lv2 = big.tile([128, 512], F32R)
    iv1 = big.tile([128, 512], F32R)
    iv2 = big.tile([128, 512], F32R)
    muiv1 = big.tile([128, 512], F32R)
    muiv2 = big.tile([128, 512], F32R)
    h11 = big.tile([128, 512], F32R)
    h12 = big.tile([128, 512], F32R)

    nc.scalar.activation(lv1, vt[:, 0:512], AF.Ln)
    nc.scalar.activation(lv2, vt[:, 512:1024], AF.Ln)

    # DVE chain, ordered by data arrival: half-1 first.
    _recip_approx_fast_f32r(nc, out=iv1, in_=vt[:, 0:512])
    # own-block stationaries (columns 0:128 are the own rows, [d, i] layout)
    mu2_own = small.tile([128, 128], F32R)  # -2 * mu own block
    nc.vector.tensor_scalar_mul(mu2_own, mt[:, 0:128], -2.0)
    sq_own = small.tile([128, 128], F32)
    nc.vector.scalar_tensor_tensor(
        out=sq_own, in0=mu2_own, scalar=0.25, in1=mu2_own,
        op0=ALU.mult, op1=ALU.mult)
    a_own = small.tile([128, 128], F32R)  # (mu^2 + var) own block
    nc.vector.tensor_add(a_own, vt[:, 0:128], sq_own)

    nc.vector.tensor_mul(muiv1, mt[:, 0:512], iv1)
    nc.vector.tensor_mul(h11, muiv1, mt[:, 0:512])
    _recip_approx_fast_f32r(nc, out=iv2, in_=vt[:, 512:1024])
    nc.vector.tensor_mul(muiv2, mt[:, 512:1024], iv2)
    nc.vector.tensor_mul(h12, muiv2, mt[:, 512:1024])

    # ---- main matmuls: R accumulated in PSUM (f32r, 1 cyc/col) ----
    exp_scr = big.tile([128, 512], F32)
    sumexp_c = small.tile([128, 2], F32)

    nc.tensor.matmul(p_R1, ones128, lv1, start=True, stop=False)
    nc.tensor.matmul(p_R1, mu2_own, muiv1, start=False, stop=False)
    nc.tensor.matmul(p_R1, a_own, iv1, start=False, stop=False)
    nc.tensor.matmul(p_R1, ones128, h11, start=False, stop=True)

    # L_own[i] = sum_d lv[d, i] over the own columns.
    nc.tensor.matmul(p_L, lv1[:, 0:128].bitcast(F32),
                     ones_col.bitcast(F32), start=True, stop=True)

    nc.scalar.activation(exp_scr, p_R1, AF.Exp, scale=C,
                         accum_out=sumexp_c[:, 0:1])
    diag_exp = small.tile([128, 1], F32)
    nc.scalar.activation(diag_exp, p_L, AF.Exp, scale=C, bias=cd_bias)

    nc.tensor.matmul(p_R2, ones128, lv2, start=True, stop=False)
    nc.tensor.matmul(p_R2, mu2_own, muiv2, start=False, stop=False)
    nc.tensor.matmul(p_R2, a_own, iv2, start=False, stop=False)
    nc.tensor.matmul(p_R2, ones128, h12, start=False, stop=True)

    # positive-pair extraction: diag of R1[:, 128:256] (pre-exp values).
    pos_scr = small.tile([128, 128], F32)
    pos_raw = small.tile([128, 1], F32)
    nc.vector.tensor_mul(pos_scr, p_R1[:, 128:256], ident)
    nc.vector.reduce_sum(pos_raw, pos_scr, axis=mybir.AxisListType.X)

    exp_scr2 = big.tile([128, 512], F32)
    nc.scalar.activation(exp_scr2, p_R2, AF.Exp, scale=C,
                         accum_out=sumexp_c[:, 1:2])

    # sumexp_adj = (half1 - self_exp) + half2
    sumexp_adj = small.tile([128, 1], F32)
    nc.vector.scalar_tensor_tensor(
        out=sumexp_adj, in0=sumexp_c[:, 0:1], scalar=diag_exp,
        in1=sumexp_c[:, 1:2], op0=ALU.subtract, op1=ALU.add)

    # loss_i = c*pos_raw - log(sumexp_adj)
    log_s = small.tile([128, 1], F32)
    nc.scalar.activation(log_s, sumexp_adj, AF.Ln)
    loss_sb = small.tile([128, 1], F32)
    nc.vector.scalar_tensor_tensor(
        out=loss_sb,
        in0=pos_raw,
        scalar=float(C),
        in1=log_s,
        op0=ALU.mult,
        op1=ALU.subtract,
    )

    # Transpose to one partition so the output DMA is a single descriptor.
    nc.tensor.transpose(p_lossT, loss_sb, ident)
    loss_row = small.tile([1, 128], F32)
    nc.vector.tensor_copy(loss_row, p_lossT)
    nc.sync.dma_start(out=loss_d[:], in_=loss_row)


# Per-core column permutation: [own 0:128 | pos block | remaining].
_P = np.concatenate([np.arange(0, 128), np.arange(512, 1024),
                     np.arange(128, 512)]).astype(np.int64)


def run_spmd(p1_loc, p2_loc, p1_scale, p2_scale, **spmd_kwargs):
    """Shard, run on 8 cores, gather.  Returns (loss_scalar, BassKernelResults)."""
    global _CACHED_NC
    mu_t = np.concatenate([p1_loc, p2_loc], axis=0).astype(np.float32).T
    var_t = np.concatenate([p1_scale, p2_scale], axis=0).astype(np.float32).T
    mu_t = np.ascontiguousarray(mu_t)    # [D, 2N]
    var_t = np.ascontiguousarray(var_t)
    if _CACHED_NC is None:
        _CACHED_NC = build_nc()
    nc = _CACHED_NC
    in_maps = []
    for c in range(N_CORES):
        cols = (_P + 128 * c) % N2
        in_maps.append({
            "muT": np.ascontiguousarray(mu_t[:, cols]),
            "varT": np.ascontiguousarray(var_t[:, cols]),
        })
    res = run_bass_kernel_spmd(nc, in_maps, core_ids=list(range(N_CORES)),
                               **spmd_kwargs)
    rows = np.concatenate([r["loss"].reshape(-1) for r in res.results])
    return np.array(rows.mean(), dtype=np.float32), res


def kernel(p1_loc, p2_loc, p1_scale, p2_scale):
    loss, _ = run_spmd(p1_loc, p2_loc, p1_scale, p2_scale)
    return loss


if __name__ == "__main__":
    import reference

    inputs = reference.setup_inputs()
    expected = np.asarray(reference.reference(**inputs))
    actual = kernel(**{k: np.asarray(v) for k, v in inputs.items()})
    rel = abs(float(actual) - float(expected)) / max(abs(float(expected)), 1e-30)
    print("expected:", expected, "actual:", actual, "rel err:", rel)
